# revision 40
# baseline (speedup 1.0000x reference)
"""Trainium2 Bass kernel for nn_CCM_73985106641118 (vq_codebook).

Data-parallel across the batch dim: core b processes batch b (8 cores, B=8).

Layout strategy: activations live feature-major ([feature chunk -> 128
partitions, tokens -> free dim]) wherever possible; each GEMM picks its
stationary operand so the output lands in the orientation its consumer needs.
Hm is produced in BOTH orientations by two GEMMs from H1T (cheaper than PE
transposes). All matmuls run fp32r (1 cyc/row when the moving free dim >= 256,
vs 4 cyc/row for fp32); fp32r operands must be produced as fp32r, so
producers write fp32r directly and non-PE readers bitcast back to f32.
Operands with a 64-sized partition dim (the K=64 cluster axis) are
zero-padded to 128 partitions (measured ~2.6x penalty for 64-dim matmuls).

Algebraic simplifications vs the reference:
 - wk_b adds a per-cluster constant to the attention scores; softmax over
   tokens is shift-invariant per row, so wk_b is dropped entirely.
 - Cluster softmax P is kept unnormalized (expP); the 1/sum factor folds
   into the Y epilogue as a per-token scale, and the bernoulli compare uses
   bern*s < expP.
 - A @ (Hm0 + b2) = A @ Hm0 + b2 because softmax rows sum to 1, so HmB is
   built bias-free and b2 is added once to C_temp.

Scheduling: the PE's HAM throttles to half speed after idle windows, so
phases are interleaved to keep the PE streaming (x-normalize/transpose fused
with GEMM1 per 512-token block; the tail runs CpT while the C norms
finalize, then theta/GT/Y per block). DMA issue queues are split: x chunks
and output stores on SP, weight loads on the Scalar DGE, small loads on the
Vector DGE. Elementwise work is spread across Scalar (activations with
accum_out for sums-of-squares), Vector, and Pool (SBUF->SBUF only; Pool has
no PSUM port).
"""

import numpy as np

import concourse.bacc as bacc
import concourse.mybir as mybir
from concourse.masks import make_identity
from concourse.tile import TileContext

f32 = mybir.dt.float32
f32r = mybir.dt.float32r
AX = mybir.AxisListType.X
OP = mybir.AluOpType
AF = mybir.ActivationFunctionType

B, N, C, H, K = 8, 2048, 512, 512, 64
NCP = N // 128   # 16 token chunks of 128
NCJ = N // 512   # 4 token chunks of 512
HC = H // 128    # 4 feature chunks of 128
SCALE = 1.0 / np.sqrt(np.float32(H))

_CACHE = {}


def s128(i):
    return slice(i * 128, (i + 1) * 128)


def s512(i):
    return slice(i * 512, (i + 1) * 512)


def build_nc():
    nc = bacc.Bacc("TRN2", target_bir_lowering=False, debug=False, num_devices=8)

    x_d = nc.dram_tensor("x", [N, C], f32, kind="ExternalInput").ap()
    bu_d = nc.dram_tensor("bern_u", [N, K], f32, kind="ExternalInput").ap()
    E_d = nc.dram_tensor("cluster_embeddings", [K, H], f32, kind="ExternalInput").ap()
    w1_d = nc.dram_tensor("mlp_w1", [C, H], f32, kind="ExternalInput").ap()
    b1_d = nc.dram_tensor("mlp_b1", [H], f32, kind="ExternalInput").ap()
    w2_d = nc.dram_tensor("mlp_w2", [H, H], f32, kind="ExternalInput").ap()
    b2_d = nc.dram_tensor("mlp_b2", [H], f32, kind="ExternalInput").ap()
    wq_d = nc.dram_tensor("wq", [H, H], f32, kind="ExternalInput").ap()
    wqb_d = nc.dram_tensor("wq_b", [H], f32, kind="ExternalInput").ap()
    wk_d = nc.dram_tensor("wk", [H, H], f32, kind="ExternalInput").ap()
    nc.dram_tensor("wk_b", [H], f32, kind="ExternalInput")  # unused (see header)
    wout_d = nc.dram_tensor("wout", [H, C], f32, kind="ExternalInput").ap()
    woutb_d = nc.dram_tensor("wout_b", [C], f32, kind="ExternalInput").ap()
    Y_d = nc.dram_tensor("Y", [N, C], f32, kind="ExternalOutput").ap()
    Co_d = nc.dram_tensor("C_out", [N, H], f32, kind="ExternalOutput").ap()

    with TileContext(nc) as tc:
        with (
            tc.tile_pool(name="big", bufs=4) as big,
            tc.tile_pool(name="med", bufs=1) as med,
            tc.tile_pool(name="sm", bufs=3) as sm,
            tc.tile_pool(name="psA", bufs=3, space="PSUM") as psA,
            tc.tile_pool(name="psT", bufs=2, space="PSUM") as psT,
            tc.tile_pool(name="psS", bufs=2, space="PSUM") as psS,
            tc.tile_pool(name="psC", bufs=1, space="PSUM") as psC,
        ):
            v = nc.vector
            sc = nc.scalar
            te = nc.tensor
            g = nc.gpsimd

            # identities first: nothing on the Pool/Vector queues ahead of
            # them, so the PE warm-up below can start at ~1.5us
            ident = med.tile([128, 128], f32, tag="ident")
            make_identity(nc, ident[:])
            identR = med.tile([128, 128], f32r, tag="rowsR")
            v.tensor_copy(identR[:], ident[:])

            # PE warm-up: the HAM throttles a cold/idle PE to 0.65-1.2 GHz;
            # stream dummy matmuls (never read) while the first x chunks are
            # still in flight so phase 1 starts at full clock
            # PE warm-up: the HAM throttles a cold/idle PE to 0.65-1.2
            # GHz; stream dummy matmuls (never read) while the first x
            # chunks are in flight so phase 1 starts at full clock. The
            # warm tile is the first psS allocation and is dead before any
            # real psS user.
            def keep_warm(n):
                # an idle PE gets HAM-throttled to half clock; dummy matmuls
                # (never read) in sparse stretches hold it at full speed. A
                # fresh psC tile per call gives safe WAR ordering vs the real
                # psC user (the Ctemp accumulator).
                w = psC.tile([128, 512], f32, tag="psC", name="warm")
                for _ in range(n):
                    te.matmul(w[:, 0:128], identR[:], identR[:],
                              start=True, stop=True)

            keep_warm(24)

            # ---- x chunk staging on the SP queue (its dedicated queue) ---
            def xstage(ncp):
                t = sm.tile([128, 512], f32, tag="xq", bufs=2, name=f"xq{ncp}")
                nc.sync.dma_start(out=t[:], in_=x_d[s128(ncp), :])
                return t

            xq_tiles = {ncp: xstage(ncp) for ncp in range(2)}

            # weight loads on the Scalar DGE queue: DMA quarters into an f32
            # stage, Pool CASTs into the f32r tile. [128, 2048]: row block q
            # of the [512, 512] weight lives at cols [q*512, (q+1)*512).
            # 3 buffers: w1,w2,wk live together; wq reuses w1's buffer after
            # GEMM1, wout reuses w2's after phase 3b (loads deferred there).
            def load_w(name):
                return med.tile([128, N], f32r, tag="W", bufs=3, name=name)

            def load_w_dma(t, dram, engs=(g, g, g, g)):
                # two half DMAs through a single stage buffer; CASTs into the
                # f32r tile are split across engines. Each load_w_dma call is
                # placed in program order right before the phase that needs
                # the weight, so the stage-buffer wait never blocks the
                # Scalar queue during head compute.
                for h in range(2):
                    st = sm.tile([128, 1024], f32, tag="wst", bufs=1,
                                 name=f"wst{h}")
                    sc.dma_start(
                        out=st[:].rearrange("p (q h) -> p q h", q=2),
                        in_=dram[h * 256:(h + 1) * 256, :]
                            .rearrange("(q p) h -> p q h", p=128))
                    for k in range(2):
                        eng = engs[h * 2 + k]
                        dst = t[:, (h * 2 + k) * 512:(h * 2 + k + 1) * 512]
                        if eng is sc:
                            sc.copy(dst, st[:, k * 512:(k + 1) * 512])
                        else:
                            eng.tensor_copy(dst, st[:, k * 512:(k + 1) * 512])
                return t

            E_f = med.tile([64, H], f32, tag="E")
            g.dma_start(out=E_f[:], in_=E_d[:, :])
            bern = med.tile([128, NCP * K], f32, tag="bern")
            g.dma_start(out=bern[:].rearrange("p (q k) -> p q k", q=16),
                         in_=bu_d.rearrange("(q p) k -> p q k", p=128))

            w1 = load_w_dma(load_w("w1"), w1_d, engs=(v, sc, v, sc))

            def bias_cols(dram, tag):
                t = med.tile([128, HC], f32, tag=tag, name=tag)
                g.dma_start(out=t[:], in_=dram.rearrange("(j p) -> p j", p=128))
                return t

            b1c = bias_cols(b1_d, "b1c")
            b2c = bias_cols(b2_d, "b2c")

            # all [1, 512] bias rows packed into one [1, 2048] tile
            rows = med.tile([1, 2048], f32, tag="rows")
            b2row = rows[0:1, 0:512]
            g.dma_start(out=b2row, in_=b2_d.rearrange("(o a) -> o a", o=1))
            wqbrow = rows[0:1, 512:1024]
            g.dma_start(out=wqbrow, in_=wqb_d.rearrange("(o a) -> o a", o=1))
            wobrow = rows[0:1, 1024:1536]
            g.dma_start(out=wobrow, in_=woutb_d.rearrange("(o a) -> o a", o=1))

            # weight slice: row block q (contraction chunk), col chunk hc
            def wsl(t, q, hc):
                return t[:, q * 512 + hc * 128: q * 512 + (hc + 1) * 128]


            ones128 = med.tile([1, 128], f32, tag="ones")
            g.memset(ones128[:], 1.0)

            def bcast_row(row, tag):
                pp = psA.tile([128, 512], f32, tag="psA", name="psA")
                te.matmul(pp[:], ones128[:], row, start=True, stop=True)
                t = med.tile([128, 512], f32, tag="bcast", bufs=2, name=tag)
                v.tensor_copy(t[:], pp[:])
                return t

            b2_bc = bcast_row(b2row, "b2bc")
            wob_bc = bcast_row(wobrow, "wobbc")
            # wqb broadcast pre-scaled by 1/sqrt(H)
            ppq = psA.tile([128, 512], f32, tag="psA", name="psA")
            te.matmul(ppq[:], ones128[:], wqbrow, start=True, stop=True)
            wqb_bc = med.tile([128, 512], f32, tag="wqbbc")
            v.tensor_scalar(wqb_bc[:], ppq[:], float(SCALE), None, OP.mult)

            # ---- E prep: norms, Ebar, padded transposes ------------------
            esq = med.tile([64, H], f32, tag="Qs", name="esq")
            ensq = med.tile([64, 1], f32, tag="ensq")
            sc.activation(esq[:], E_f[:], AF.Square, accum_out=ensq[:])
            enrm = med.tile([64, 1], f32, tag="enrm")
            sc.sqrt(enrm[:], ensq[:])
            einv = med.tile([64, 1], f32, tag="einv")
            v.reciprocal(einv[:], enrm[:])
            Ebar = med.tile([64, H], f32, tag="Ebar")
            v.tensor_scalar(Ebar[:], E_f[:], einv[:], None, OP.mult)

            # E_rPad: [128, 512] E on top, zeros below (theta stationary)
            E_rP = med.tile([128, H], f32r, tag="ErP")
            g.memset(E_rP[64:128, :].bitcast(f32), 0.0)
            g.tensor_copy(E_rP[0:64, :], E_f[:])

            # EbarT / ET chunks padded to [128, 128] (zero cols 64..127)
            EbarT, ETp = [], []
            for hc in range(HC):
                t = med.tile([128, 128], f32r, tag=f"ebt{hc}", name=f"ebt{hc}")
                g.memset(t[:, 64:128].bitcast(f32), 0.0)
                pt = psT.tile([128, 512], f32, tag="pt512")
                te.transpose(pt[0:128, 0:64], Ebar[:, s128(hc)], ident[0:64, 0:64])
                sc.copy(t[:, 0:64], pt[0:128, 0:64])
                EbarT.append(t)
                t2 = med.tile([128, 128], f32r, tag=f"et{hc}", name=f"et{hc}")
                g.memset(t2[:, 64:128].bitcast(f32), 0.0)
                pt2 = psT.tile([128, 512], f32, tag="pt512")
                te.transpose(pt2[0:128, 0:64], E_f[:, s128(hc)], ident[0:64, 0:64])
                sc.copy(t2[:, 0:64], pt2[0:128, 0:64])
                ETp.append(t2)

            # ---- phase 1+2 fused per 512-token block: l2norm + transpose,
            # then H1T = relu(w1.T @ xnT + b1) for the block  (f32r) -------
            # xnT_all[:, cc*2048 + n] holds feature chunk cc, token n
            xnT_all = big.tile([128, 4 * N], f32r, tag="bigX", bufs=1,
                               name="xnT_all")

            def xnT(cc):
                return xnT_all[:, cc * N:(cc + 1) * N]

            H1T = [big.tile([128, N], f32r, tag="B", name=f"H1T{i}") for i in range(HC)]
            for j in range(NCJ):
                for q in range(4):
                    ncp = j * 4 + q
                    xq = xq_tiles[ncp] if ncp < 2 else xstage(ncp)
                    xt = xq[:]
                    xsq = sm.tile([128, C], f32, tag="xsq", bufs=1, name="xsq")
                    ssq = sm.tile([128, 1], f32, tag="ssq", bufs=2, name="ssq")
                    sc.activation(xsq[:], xt, AF.Square, accum_out=ssq[:])
                    nrm = sm.tile([128, 1], f32, tag="nrm", bufs=2, name="nrm")
                    sc.sqrt(nrm[:], ssq[:])
                    nrm2 = sm.tile([128, 1], f32, tag="nrm2", bufs=2, name="nrm2")
                    v.tensor_scalar(nrm2[:], nrm[:], 1e-12, None, OP.max)
                    inv = sm.tile([128, 1], f32, tag="inv", bufs=2, name="inv")
                    v.reciprocal(inv[:], nrm2[:])
                    xn = sm.tile([128, C], f32r, tag="xn", bufs=2, name="xn")
                    v.tensor_scalar(xn[:], xt, inv[:], None, OP.mult)
                    # 4 transposes into one PSUM bank, one strided copy out
                    pt = psT.tile([128, 512], f32r, tag="pt512")
                    for cc in range(HC):
                        te.transpose(pt[:, s128(cc)], xn[:, s128(cc)],
                                     identR[:])
                    dst = xnT_all[:].rearrange(
                        "p (c n) -> p c n", c=4)[:, :, ncp * 128:(ncp + 1) * 128]
                    if ncp % 2 == 0:
                        v.tensor_copy(dst, pt[:].rearrange("p (c n) -> p c n", c=4))
                    else:
                        sc.copy(dst, pt[:].rearrange("p (c n) -> p c n", c=4))
                # GEMM1 for this 512-token block
                keep_warm(2)
                for h1c in range(HC):
                    pp = psA.tile([128, 512], f32, tag="psA", name="psA")
                    for cc in range(HC):
                        te.matmul(pp[:], wsl(w1, cc, h1c),
                                  xnT(cc)[:, s512(j)],
                                  start=(cc == 0), stop=(cc == HC - 1))
                    sc.activation(H1T[h1c][:, s512(j)], pp[:], AF.Relu,
                                  bias=b1c[:, h1c:h1c + 1], scale=1.0)


            # w2 load deferred here: its stage wait no longer blocks the
            # Scalar queue during the head
            w2 = load_w_dma(load_w("w2"), w2_d)

            # ---- phase 3a: HmT = w2.T @ H1T + b2 (feature-major, f32r) ---
            HmT = [big.tile([128, N], f32r, tag="C", name=f"HmT{i}") for i in range(HC)]
            for hc in range(HC):
                for ncj in range(NCJ):
                    pp = psA.tile([128, 512], f32, tag="psA", name="psA")
                    for q in range(HC):
                        te.matmul(pp[:], wsl(w2, q, hc),
                                  H1T[q][:, s512(ncj)],
                                  start=(q == 0), stop=(q == HC - 1))
                    if ncj % 2 == 0:
                        v.tensor_scalar(HmT[hc][:, s512(ncj)], pp[:],
                                        b2c[:, hc:hc + 1], None, OP.add)
                    else:
                        sc.activation(HmT[hc][:, s512(ncj)], pp[:], AF.Identity,
                                      bias=b2c[:, hc:hc + 1], scale=1.0)

            wk_w = load_w_dma(load_w("wk"), wk_d)

            # ---- phase 3b: HmB = H1 @ w2 (token-major, NO bias; f32r) ----
            # b2 is added to C_temp instead (softmax rows sum to 1).
            HmB_all = big.tile([128, 4 * N], f32r, tag="bigX", bufs=1,
                               name="HmB_all")
            for ncp in range(NCP):
                pp = psA.tile([128, 512], f32, tag="psA", name="psA")
                for q in range(HC):
                    te.matmul(pp[:], H1T[q][:, s128(ncp)],
                              w2[:, s512(q)],
                              start=(q == 0), stop=(q == HC - 1))
                dst = HmB_all[:, ncp * 512:(ncp + 1) * 512]
                if ncp % 2 == 0:
                    v.tensor_copy(dst, pp[:])
                else:
                    sc.copy(dst, pp[:])

            wq_w = load_w_dma(load_w("wq"), wq_d)

            # ---- phase 4: logitsT -> expT (=PT), expP, M, MT, inv_s ------
            PT = big.tile([128, N], f32r, tag="B", name="PT")
            g.memset(PT[64:128, :].bitcast(f32), 0.0)
            MT = big.tile([128, N], f32r, tag="B", name="MT")
            g.memset(MT[64:128, :].bitcast(f32), 0.0)
            inv_s = med.tile([128, NCP], f32, tag="invs")
            for ncj in range(NCJ):
                pl = psS.tile([128, 512], f32, tag="psS", name="psS")
                for hc in range(HC):
                    te.matmul(pl[:], EbarT[hc][:],
                              HmT[hc][:, s512(ncj)],
                              start=(hc == 0), stop=(hc == HC - 1))
                sc.activation(PT[0:64, s512(ncj)], pl[0:64, :], AF.Exp)
                mtp = psT.tile([128, 512], f32r, tag="pt512")
                for q in range(4):
                    ncp = ncj * 4 + q
                    # expP (token-major) via transpose of exp'd PT chunk
                    ep = psT.tile([128, 512], f32r, tag="pt512")
                    te.transpose(ep[:, 0:128], PT[:, s128(ncp)], identR[:])
                    s_col = sm.tile([128, 1], f32, tag="scol", bufs=2, name="scol")
                    v.reduce_sum(s_col[:], ep[:, 0:128].bitcast(f32), axis=AX)
                    v.reciprocal(inv_s[:, ncp:ncp + 1], s_col[:])
                    bs = sm.tile([128, K], f32, tag="bs", bufs=2, name="bs")
                    v.tensor_scalar(bs[:], bern[:, ncp * K:(ncp + 1) * K],
                                    s_col[:], None, OP.mult)
                    M = sm.tile([128, K], f32r, tag="M", bufs=2, name="M")
                    v.tensor_tensor(M[:], ep[:, 0:64].bitcast(f32), bs[:],
                                    OP.is_gt)
                    te.transpose(mtp[0:64, s128(q)], M[:], identR[:])
                if ncj % 2 == 0:
                    v.tensor_copy(MT[0:64, s512(ncj)], mtp[0:64, :])
                else:
                    sc.copy(MT[0:64, s512(ncj)], mtp[0:64, :])

            wout_w = load_w_dma(load_w("wout"), wout_d)

            # ---- phase 5: wkT (wk transposed); Kmat itself is never
            # materialized: scores = Q @ (Hm wk)^T = (Q wk^T) @ Hm^T -------
            wkT = load_w("wkT")
            for hc in range(HC):
                pt = psT.tile([128, 512], f32r, tag="pt512")
                for q in range(HC):
                    te.transpose(pt[:, s128(q)], wsl(wk_w, q, hc), identR[:])
                if hc % 2 == 0:
                    v.tensor_copy(wkT[:, hc * 512:(hc + 1) * 512], pt[:])
                else:
                    sc.copy(wkT[:, hc * 512:(hc + 1) * 512], pt[:])

            # ---- phase 6: Q (k-major, padded) -> QT chunks ---------------
            pq = psS.tile([128, 512], f32, tag="psS", name="psS")
            for q in range(HC):
                te.matmul(pq[:], ETp[q][:],
                          wq_w[:, s512(q)],
                          start=(q == 0), stop=(q == HC - 1))
            keep_warm(4)
            Qs = med.tile([128, 512], f32r, tag="Qs", name="Qs")
            v.scalar_tensor_tensor(Qs[:], pq[:], float(SCALE), wqb_bc[:],
                                   OP.mult, OP.add)
            QT = []
            for hc in range(HC):
                ptq = psT.tile([128, 512], f32r, tag="pt512")
                te.transpose(ptq[:, 0:128], Qs[:, s128(hc)], identR[:])
                t = med.tile([128, 128], f32r, tag=f"qt{hc}", name=f"qt{hc}")
                sc.copy(t[:], ptq[:, 0:128])
                QT.append(t)
            # Q2 = Q @ wk^T (k-major, padded rows), then Q2T chunks
            pq2 = psS.tile([128, 512], f32, tag="psS", name="psS")
            for hc in range(HC):
                te.matmul(pq2[:], QT[hc][:], wkT[:, hc * 512:(hc + 1) * 512],
                          start=(hc == 0), stop=(hc == HC - 1))
            keep_warm(4)
            Q2s = med.tile([128, 512], f32r, tag="Qs", name="Q2s")
            v.tensor_copy(Q2s[:], pq2[:])
            Q2T = []
            for hc in range(HC):
                ptq2 = psT.tile([128, 512], f32r, tag="pt512")
                te.transpose(ptq2[:, 0:128], Q2s[:, s128(hc)], identR[:])
                t2q = med.tile([128, 128], f32r, tag=f"qt{hc}", name=f"q2t{hc}")
                sc.copy(t2q[:], ptq2[:, 0:128])
                Q2T.append(t2q)

            # ---- phase 7: scores -> expS (zero-padded rows), row sums ----
            expS = big.tile([128, N], f32r, tag="B", name="expS")
            g.memset(expS[64:128, :].bitcast(f32), 0.0)
            pses = []
            for ncj in range(NCJ):
                ps_ = psS.tile([128, 512], f32, tag="psS", name="psS")
                for hc in range(HC):
                    te.matmul(ps_[:], Q2T[hc][:],
                              HmT[hc][:, s512(ncj)],
                              start=(hc == 0), stop=(hc == HC - 1))
                keep_warm(3)
                pse = med.tile([64, 1], f32, tag=f"pse{ncj}", name=f"pse{ncj}")
                sc.activation(expS[0:64, s512(ncj)], ps_[0:64, :], AF.Exp,
                              accum_out=pse[:])
                pses.append(pse)
            sA = med.tile([64, 1], f32, tag="sA")
            v.tensor_tensor(sA[:], pses[0][:], pses[1][:], OP.add)
            sA2 = med.tile([64, 1], f32, tag="sA2")
            v.tensor_tensor(sA2[:], pses[2][:], pses[3][:], OP.add)
            sA3 = med.tile([64, 1], f32, tag="sA3")
            v.tensor_tensor(sA3[:], sA[:], sA2[:], OP.add)
            invA = med.tile([64, 1], f32, tag="invA")
            v.reciprocal(invA[:], sA3[:])

            # expST: token-major chunks [128 tok, 128 kpad] (zero right cols)
            expST = big.tile([128, NCP * 128], f32r, tag="B", name="expST")
            for jj in range(4):
                pt = psT.tile([128, 512], f32r, tag="pt512")
                for q in range(4):
                    te.transpose(pt[:, s128(q)], expS[:, s128(jj * 4 + q)],
                                 identR[:])
                if jj % 2 == 0:
                    v.tensor_copy(expST[:, s512(jj)], pt[:])
                else:
                    sc.copy(expST[:, s512(jj)], pt[:])

            # ---- phase 8: Ctemp = (A @ Hm0) * invA + b2  (padded k) ------
            pc = psC.tile([128, 512], f32, tag="psC", name="psC")
            for ncp in range(NCP):
                te.matmul(pc[:], expST[:, s128(ncp)],
                          HmB_all[:, ncp * 512:(ncp + 1) * 512],
                          start=(ncp == 0), stop=(ncp == NCP - 1))
            Ctp = med.tile([128, H], f32r, tag="bern", name="Ctp")
            g.memset(Ctp[64:128, :].bitcast(f32), 0.0)
            v.scalar_tensor_tensor(Ctp[0:64, :], pc[0:64, :], invA[:],
                                   b2_bc[0:64, :], OP.mult, OP.add)

            # ---- phase 9: CpreT (+ norms on Scalar), then CpT right away
            # (CpT uses unscaled Ctp; the invn_bc scale rides the psum->cb
            # copy, so the PE keeps streaming while the norms finalize) ----
            CpreT = [big.tile([128, N], f32r, tag="D", name=f"CpreT{i}")
                     for i in range(HC)]
            invn = []
            for hc in range(HC):
                parts = []
                for ncj in range(NCJ):
                    pp = psA.tile([128, 512], f32, tag="psA", name="psA")
                    te.matmul(pp[:], Ctp[:, s128(hc)],
                              MT[:, s512(ncj)],
                              start=True, stop=True)
                    if ncj % 2 == 0:
                        v.tensor_copy(CpreT[hc][:, s512(ncj)], pp[:])
                    else:
                        sc.copy(CpreT[hc][:, s512(ncj)], pp[:])
                    sqs = sm.tile([128, 512], f32, tag="xsq", bufs=1, name="sqs")
                    prt = sm.tile([128, 1], f32, tag="cprt", bufs=4, name="cprt")
                    sc.activation(sqs[:], pp[:], AF.Square, accum_out=prt[:])
                    parts.append(prt)
                keep_warm(3)
                pa = sm.tile([128, 1], f32, tag="cpa", bufs=2, name="cpa")
                v.tensor_tensor(pa[:], parts[0][:], parts[1][:], OP.add)
                pb = sm.tile([128, 1], f32, tag="cpb", bufs=2, name="cpb")
                v.tensor_tensor(pb[:], parts[2][:], parts[3][:], OP.add)
                csq = med.tile([128, 1], f32, tag=f"csq{hc}", name=f"csq{hc}")
                v.tensor_tensor(csq[:], pa[:], pb[:], OP.add)
                cn = med.tile([128, 1], f32, tag=f"cn{hc}", name=f"cn{hc}")
                sc.sqrt(cn[:], csq[:])
                cn2 = med.tile([128, 1], f32, tag=f"cn2{hc}", name=f"cn2{hc}")
                v.tensor_scalar(cn2[:], cn[:], 1e-12, None, OP.max)
                iv = med.tile([128, 1], f32, tag=f"ivn{hc}", name=f"ivn{hc}")
                v.reciprocal(iv[:], cn2[:])
                invn.append(iv)

            # invn as broadcast tile for the token-major C scaling
            invnrow = rows[0:1, 1536:2048]
            for hc in range(HC):
                pt = psT.tile([128, 512], f32, tag="pt512")
                te.matmul(pt[0:1, 0:128], invn[hc][:, 0:1], ident[:],
                          start=True, stop=True)
                v.tensor_copy(invnrow[0:1, s128(hc)], pt[0:1, 0:128])
            ppn = psA.tile([128, 512], f32, tag="psA", name="psA")
            te.matmul(ppn[:], ones128[:], invnrow, start=True, stop=True)
            invn_bc = med.tile([128, H], f32, tag="wqbbc", name="invnbc")
            v.tensor_copy(invn_bc[:], ppn[:])

            # CpT (token-major) -> scaled C output, batched 1MB stores
            # staging reuses the big-A buffers (HmB is dead after phase 8)
            cb_all = big.tile([128, 4 * N], f32, tag="bigX", bufs=1,
                              name="cb_all")
            for j in range(NCJ):
                cb = cb_all[:, j * N:(j + 1) * N]
                for q in range(4):
                    ncp = j * 4 + q
                    pp = psA.tile([128, 512], f32, tag="psA", name="psA")
                    te.matmul(pp[:], MT[:, s128(ncp)],
                              Ctp[:], start=True, stop=True)
                    if q % 2 == 0:
                        v.tensor_tensor(cb[:, s512(q)], pp[:], invn_bc[:],
                                        OP.mult)
                    else:
                        sc.copy(cb[:, s512(q)], pp[:])
                        g.tensor_tensor(cb[:, s512(q)], cb[:, s512(q)],
                                        invn_bc[:], OP.mult)
                nc.sync.dma_start(
                    out=Co_d[s512(j), :].rearrange("(q p) c -> p q c", p=128),
                    in_=cb[:].rearrange("p (q c) -> p q c", q=4))

            # ---- phase 10: per-chunk fused tail: theta -> GT -> Y --------
            # thetaT_pre = E.T @ expT (unnormalized); GT = (CpreT*invn + HmT)
            # * thetaT_pre in place; Y = (GT.T @ wout) * inv_s + wout_b
            yb_all = big.tile([128, 4 * N], f32, tag="bigX", bufs=1,
                              name="yb_all")
            for j in range(NCJ):
                for hc in range(HC):
                    pth = psS.tile([128, 512], f32, tag="psS", name="psS")
                    te.matmul(pth[:], E_rP[:, s128(hc)],
                              PT[:, s512(j)], start=True, stop=True)
                    gslice = CpreT[hc][:, s512(j)]
                    ct = sm.tile([128, 512], f32, tag="ct", bufs=2, name="ct")
                    sc.activation(ct[:], gslice.bitcast(f32), AF.Identity,
                                  bias=0.0, scale=invn[hc][:])
                    if hc % 2 == 0:
                        v.tensor_tensor(ct[:], ct[:],
                                        HmT[hc][:, s512(j)].bitcast(f32),
                                        OP.add)
                    else:
                        g.tensor_tensor(ct[:], ct[:],
                                        HmT[hc][:, s512(j)].bitcast(f32),
                                        OP.add)
                    v.tensor_tensor(gslice, ct[:], pth[:], OP.mult)
                keep_warm(3)
                yb = yb_all[:, j * N:(j + 1) * N]
                for q in range(4):
                    ncp = j * 4 + q
                    pp = psA.tile([128, 512], f32, tag="psA", name="psA")
                    for hc in range(HC):
                        te.matmul(pp[:], CpreT[hc][:, s128(ncp)],
                                  wout_w[:, s512(hc)],
                                  start=(hc == 0), stop=(hc == HC - 1))
                    if q % 2 == 0:
                        v.scalar_tensor_tensor(yb[:, s512(q)], pp[:],
                                               inv_s[:, ncp:ncp + 1],
                                               wob_bc[:], OP.mult, OP.add)
                    else:
                        sc.activation(yb[:, s512(q)], pp[:], AF.Identity,
                                      bias=0.0, scale=inv_s[:, ncp:ncp + 1])
                        g.tensor_tensor(yb[:, s512(q)], yb[:, s512(q)],
                                        wob_bc[:], OP.add)
                nc.sync.dma_start(
                    out=Y_d[s512(j), :].rearrange("(q p) c -> p q c", p=128),
                    in_=yb[:].rearrange("p (q c) -> p q c", q=4))

    nc.finalize()
    return nc


def _get_nc():
    if "nc" not in _CACHE:
        _CACHE["nc"] = build_nc()
    return _CACHE["nc"]


def kernel(**inputs):
    from concourse.bass_utils import run_bass_kernel_spmd

    nc = _get_nc()
    arr = {k: np.ascontiguousarray(np.asarray(v, dtype=np.float32))
           for k, v in inputs.items()}
    shared = {k: arr[k] for k in
              ("cluster_embeddings", "mlp_w1", "mlp_b1", "mlp_w2", "mlp_b2",
               "wq", "wq_b", "wk", "wk_b", "wout", "wout_b")}
    in_maps = [dict(x=arr["x"][b], bern_u=arr["bern_u"][b], **shared)
               for b in range(B)]
    res = run_bass_kernel_spmd(nc, in_maps, list(range(B))).results
    Y = np.stack([res[b]["Y"] for b in range(B)])
    Co = np.stack([res[b]["C_out"] for b in range(B)])
    return (Y, Co)


if __name__ == "__main__":
    import os
    os.environ.setdefault("JAX_PLATFORMS", "cpu")
    from concourse.timeline_sim import TimelineSim

    nc = build_nc()
    ts = TimelineSim(nc, trace=False)
    print("TimelineSim:", ts.simulate(), "ns")


# revision 43
# speedup vs baseline: 1.1965x; 1.1965x over previous
"""Trainium2 Bass kernel for nn_CCM_73985106641118 (vq_codebook).

Data-parallel across the batch dim: core b processes batch b (8 cores, B=8).

Layout strategy: activations live feature-major ([feature chunk -> 128
partitions, tokens -> free dim]) wherever possible; each GEMM picks its
stationary operand so the output lands in the orientation its consumer needs.
Hm is produced in BOTH orientations by two GEMMs from H1T (cheaper than PE
transposes). All matmuls run fp32r (1 cyc/row when the moving free dim >= 256,
vs 4 cyc/row for fp32); fp32r operands must be produced as fp32r, so
producers write fp32r directly and non-PE readers bitcast back to f32.
Operands with a 64-sized partition dim (the K=64 cluster axis) are
zero-padded to 128 partitions (measured ~2.6x penalty for 64-dim matmuls).

Algebraic simplifications vs the reference:
 - wk_b adds a per-cluster constant to the attention scores; softmax over
   tokens is shift-invariant per row, so wk_b is dropped entirely.
 - Cluster softmax P is kept unnormalized (expP); the 1/sum factor folds
   into the Y epilogue as a per-token scale, and the bernoulli compare uses
   bern*s < expP.
 - A @ (Hm0 + b2) = A @ Hm0 + b2 because softmax rows sum to 1, so HmB is
   built bias-free and b2 is added once to C_temp.

Scheduling: the PE's HAM throttles to half speed after idle windows, so
phases are interleaved to keep the PE streaming (x-normalize/transpose fused
with GEMM1 per 512-token block; the tail runs CpT while the C norms
finalize, then theta/GT/Y per block). DMA issue queues are split: x chunks
and output stores on SP, weight loads on the Scalar DGE, small loads on the
Vector DGE. Elementwise work is spread across Scalar (activations with
accum_out for sums-of-squares), Vector, and Pool (SBUF->SBUF only; Pool has
no PSUM port).
"""

import numpy as np

import concourse.bacc as bacc
import concourse.mybir as mybir
from concourse.masks import make_identity
from concourse.tile import TileContext

f32 = mybir.dt.float32
f32r = mybir.dt.float32r
AX = mybir.AxisListType.X
OP = mybir.AluOpType
AF = mybir.ActivationFunctionType

B, N, C, H, K = 8, 2048, 512, 512, 64
NCP = N // 128   # 16 token chunks of 128
NCJ = N // 512   # 4 token chunks of 512
HC = H // 128    # 4 feature chunks of 128
SCALE = 1.0 / np.sqrt(np.float32(H))

_CACHE = {}


def s128(i):
    return slice(i * 128, (i + 1) * 128)


def s512(i):
    return slice(i * 512, (i + 1) * 512)


def build_nc():
    nc = bacc.Bacc("TRN2", target_bir_lowering=False, debug=False, num_devices=8)

    x_d = nc.dram_tensor("x", [N, C], f32, kind="ExternalInput").ap()
    bu_d = nc.dram_tensor("bern_u", [N, K], f32, kind="ExternalInput").ap()
    E_d = nc.dram_tensor("cluster_embeddings", [K, H], f32, kind="ExternalInput").ap()
    w1_d = nc.dram_tensor("mlp_w1", [C, H], f32, kind="ExternalInput").ap()
    b1_d = nc.dram_tensor("mlp_b1", [H], f32, kind="ExternalInput").ap()
    w2_d = nc.dram_tensor("mlp_w2", [H, H], f32, kind="ExternalInput").ap()
    b2_d = nc.dram_tensor("mlp_b2", [H], f32, kind="ExternalInput").ap()
    wq_d = nc.dram_tensor("wq", [H, H], f32, kind="ExternalInput").ap()
    wqb_d = nc.dram_tensor("wq_b", [H], f32, kind="ExternalInput").ap()
    wk_d = nc.dram_tensor("wk", [H, H], f32, kind="ExternalInput").ap()
    nc.dram_tensor("wk_b", [H], f32, kind="ExternalInput")  # unused (see header)
    wout_d = nc.dram_tensor("wout", [H, C], f32, kind="ExternalInput").ap()
    woutb_d = nc.dram_tensor("wout_b", [C], f32, kind="ExternalInput").ap()
    Y_d = nc.dram_tensor("Y", [N, C], f32, kind="ExternalOutput").ap()
    Co_d = nc.dram_tensor("C_out", [N, H], f32, kind="ExternalOutput").ap()

    with TileContext(nc) as tc:
        with (
            tc.tile_pool(name="big", bufs=4) as big,
            tc.tile_pool(name="med", bufs=1) as med,
            tc.tile_pool(name="sm", bufs=3) as sm,
            tc.tile_pool(name="psA", bufs=3, space="PSUM") as psA,
            tc.tile_pool(name="psT", bufs=2, space="PSUM") as psT,
            tc.tile_pool(name="psS", bufs=2, space="PSUM") as psS,
            tc.tile_pool(name="psC", bufs=1, space="PSUM") as psC,
        ):
            v = nc.vector
            sc = nc.scalar
            te = nc.tensor
            g = nc.gpsimd

            # identities first: nothing on the Pool/Vector queues ahead of
            # them, so the PE warm-up below can start at ~1.5us
            ident = med.tile([128, 128], f32, tag="ident")
            make_identity(nc, ident[:])
            identR = med.tile([128, 128], f32r, tag="rowsR")
            v.tensor_copy(identR[:], ident[:])

            # PE warm-up: the HAM throttles a cold/idle PE to 0.65-1.2 GHz;
            # stream dummy matmuls (never read) while the first x chunks are
            # still in flight so phase 1 starts at full clock
            # PE warm-up: the HAM throttles a cold/idle PE to 0.65-1.2
            # GHz; stream dummy matmuls (never read) while the first x
            # chunks are in flight so phase 1 starts at full clock. The
            # warm tile is the first psS allocation and is dead before any
            # real psS user.
            warm = psS.tile([128, 512], f32, tag="psS", name="warm")
            for _ in range(24):
                te.matmul(warm[:, 0:128], identR[:], identR[:],
                          start=True, stop=True)

            # ---- x chunk staging on the SP queue (its dedicated queue) ---
            def xstage(ncp):
                t = sm.tile([128, 512], f32, tag="xq", bufs=2, name=f"xq{ncp}")
                nc.sync.dma_start(out=t[:], in_=x_d[s128(ncp), :])
                return t

            xq_tiles = {ncp: xstage(ncp) for ncp in range(2)}

            # weight loads on the Scalar DGE queue: DMA quarters into an f32
            # stage, Pool CASTs into the f32r tile. [128, 2048]: row block q
            # of the [512, 512] weight lives at cols [q*512, (q+1)*512).
            # 3 buffers: w1,w2,wk live together; wq reuses w1's buffer after
            # GEMM1, wout reuses w2's after phase 3b (loads deferred there).
            def load_w(name):
                return med.tile([128, N], f32r, tag="W", bufs=3, name=name)

            def load_w_dma(t, dram, engs=(g, g, g, g)):
                # two half DMAs through a single stage buffer; CASTs into the
                # f32r tile are split across engines. Each load_w_dma call is
                # placed in program order right before the phase that needs
                # the weight, so the stage-buffer wait never blocks the
                # Scalar queue during head compute.
                for h in range(2):
                    st = sm.tile([128, 1024], f32, tag="wst", bufs=1,
                                 name=f"wst{h}")
                    sc.dma_start(
                        out=st[:].rearrange("p (q h) -> p q h", q=2),
                        in_=dram[h * 256:(h + 1) * 256, :]
                            .rearrange("(q p) h -> p q h", p=128))
                    for k in range(2):
                        eng = engs[h * 2 + k]
                        dst = t[:, (h * 2 + k) * 512:(h * 2 + k + 1) * 512]
                        if eng is sc:
                            sc.copy(dst, st[:, k * 512:(k + 1) * 512])
                        else:
                            eng.tensor_copy(dst, st[:, k * 512:(k + 1) * 512])
                return t

            E_f = med.tile([64, H], f32, tag="E")
            g.dma_start(out=E_f[:], in_=E_d[:, :])
            bern = med.tile([128, NCP * K], f32, tag="bern")
            g.dma_start(out=bern[:].rearrange("p (q k) -> p q k", q=16),
                         in_=bu_d.rearrange("(q p) k -> p q k", p=128))

            w1 = load_w_dma(load_w("w1"), w1_d, engs=(v, sc, v, sc))

            def bias_cols(dram, tag):
                t = med.tile([128, HC], f32, tag=tag, name=tag)
                g.dma_start(out=t[:], in_=dram.rearrange("(j p) -> p j", p=128))
                return t

            b1c = bias_cols(b1_d, "b1c")
            b2c = bias_cols(b2_d, "b2c")

            # all [1, 512] bias rows packed into one [1, 2048] tile
            rows = med.tile([1, 2048], f32, tag="rows")
            b2row = rows[0:1, 0:512]
            g.dma_start(out=b2row, in_=b2_d.rearrange("(o a) -> o a", o=1))
            wqbrow = rows[0:1, 512:1024]
            g.dma_start(out=wqbrow, in_=wqb_d.rearrange("(o a) -> o a", o=1))
            wobrow = rows[0:1, 1024:1536]
            g.dma_start(out=wobrow, in_=woutb_d.rearrange("(o a) -> o a", o=1))

            # weight slice: row block q (contraction chunk), col chunk hc
            def wsl(t, q, hc):
                return t[:, q * 512 + hc * 128: q * 512 + (hc + 1) * 128]


            ones128 = med.tile([1, 128], f32, tag="ones")
            g.memset(ones128[:], 1.0)

            def bcast_row(row, tag):
                pp = psA.tile([128, 512], f32, tag="psA", name="psA")
                te.matmul(pp[:], ones128[:], row, start=True, stop=True)
                t = med.tile([128, 512], f32, tag="bcast", bufs=2, name=tag)
                v.tensor_copy(t[:], pp[:])
                return t

            b2_bc = bcast_row(b2row, "b2bc")
            wob_bc = bcast_row(wobrow, "wobbc")
            # wqb broadcast pre-scaled by 1/sqrt(H)
            ppq = psA.tile([128, 512], f32, tag="psA", name="psA")
            te.matmul(ppq[:], ones128[:], wqbrow, start=True, stop=True)
            wqb_bc = med.tile([128, 512], f32, tag="wqbbc")
            v.tensor_scalar(wqb_bc[:], ppq[:], float(SCALE), None, OP.mult)

            # ---- E prep: norms, Ebar, padded transposes ------------------
            esq = med.tile([64, H], f32, tag="Qs", name="esq")
            ensq = med.tile([64, 1], f32, tag="ensq")
            sc.activation(esq[:], E_f[:], AF.Square, accum_out=ensq[:])
            enrm = med.tile([64, 1], f32, tag="enrm")
            sc.sqrt(enrm[:], ensq[:])
            einv = med.tile([64, 1], f32, tag="einv")
            v.reciprocal(einv[:], enrm[:])
            Ebar = med.tile([64, H], f32, tag="Ebar")
            v.tensor_scalar(Ebar[:], E_f[:], einv[:], None, OP.mult)

            # E_rPad: [128, 512] E on top, zeros below (theta stationary)
            E_rP = med.tile([128, H], f32r, tag="ErP")
            g.memset(E_rP[64:128, :].bitcast(f32), 0.0)
            g.tensor_copy(E_rP[0:64, :], E_f[:])

            # EbarT / ET chunks padded to [128, 128] (zero cols 64..127)
            EbarT, ETp = [], []
            for hc in range(HC):
                t = med.tile([128, 128], f32r, tag=f"ebt{hc}", name=f"ebt{hc}")
                g.memset(t[:, 64:128].bitcast(f32), 0.0)
                pt = psT.tile([128, 512], f32, tag="pt512")
                te.transpose(pt[0:128, 0:64], Ebar[:, s128(hc)], ident[0:64, 0:64])
                sc.copy(t[:, 0:64], pt[0:128, 0:64])
                EbarT.append(t)
                t2 = med.tile([128, 128], f32r, tag=f"et{hc}", name=f"et{hc}")
                g.memset(t2[:, 64:128].bitcast(f32), 0.0)
                pt2 = psT.tile([128, 512], f32, tag="pt512")
                te.transpose(pt2[0:128, 0:64], E_f[:, s128(hc)], ident[0:64, 0:64])
                sc.copy(t2[:, 0:64], pt2[0:128, 0:64])
                ETp.append(t2)

            # ---- phase 1+2 fused per 512-token block: l2norm + transpose,
            # then H1T = relu(w1.T @ xnT + b1) for the block  (f32r) -------
            # xnT_all[:, cc*2048 + n] holds feature chunk cc, token n
            xnT_all = big.tile([128, 4 * N], f32r, tag="bigX", bufs=1,
                               name="xnT_all")

            def xnT(cc):
                return xnT_all[:, cc * N:(cc + 1) * N]

            H1T = [big.tile([128, N], f32r, tag="B", name=f"H1T{i}") for i in range(HC)]
            for j in range(NCJ):
                for q in range(4):
                    ncp = j * 4 + q
                    xq = xq_tiles[ncp] if ncp < 2 else xstage(ncp)
                    xt = xq[:]
                    xsq = sm.tile([128, C], f32, tag="xsq", bufs=1, name="xsq")
                    ssq = sm.tile([128, 1], f32, tag="ssq", bufs=2, name="ssq")
                    sc.activation(xsq[:], xt, AF.Square, accum_out=ssq[:])
                    nrm = sm.tile([128, 1], f32, tag="nrm", bufs=2, name="nrm")
                    sc.sqrt(nrm[:], ssq[:])
                    nrm2 = sm.tile([128, 1], f32, tag="nrm2", bufs=2, name="nrm2")
                    v.tensor_scalar(nrm2[:], nrm[:], 1e-12, None, OP.max)
                    inv = sm.tile([128, 1], f32, tag="inv", bufs=2, name="inv")
                    v.reciprocal(inv[:], nrm2[:])
                    xn = sm.tile([128, C], f32r, tag="xn", bufs=2, name="xn")
                    v.tensor_scalar(xn[:], xt, inv[:], None, OP.mult)
                    # 4 transposes into one PSUM bank, one strided copy out
                    pt = psT.tile([128, 512], f32r, tag="pt512")
                    for cc in range(HC):
                        te.transpose(pt[:, s128(cc)], xn[:, s128(cc)],
                                     identR[:])
                    dst = xnT_all[:].rearrange(
                        "p (c n) -> p c n", c=4)[:, :, ncp * 128:(ncp + 1) * 128]
                    if ncp % 2 == 0:
                        v.tensor_copy(dst, pt[:].rearrange("p (c n) -> p c n", c=4))
                    else:
                        sc.copy(dst, pt[:].rearrange("p (c n) -> p c n", c=4))
                # GEMM1 for this 512-token block
                for h1c in range(HC):
                    pp = psA.tile([128, 512], f32, tag="psA", name="psA")
                    for cc in range(HC):
                        te.matmul(pp[:], wsl(w1, cc, h1c),
                                  xnT(cc)[:, s512(j)],
                                  start=(cc == 0), stop=(cc == HC - 1))
                    sc.activation(H1T[h1c][:, s512(j)], pp[:], AF.Relu,
                                  bias=b1c[:, h1c:h1c + 1], scale=1.0)


            # w2 load deferred here: its stage wait no longer blocks the
            # Scalar queue during the head
            w2 = load_w_dma(load_w("w2"), w2_d)

            # ---- phase 3a: HmT = w2.T @ H1T + b2 (feature-major, f32r) ---
            HmT = [big.tile([128, N], f32r, tag="C", name=f"HmT{i}") for i in range(HC)]
            for hc in range(HC):
                for ncj in range(NCJ):
                    pp = psA.tile([128, 512], f32, tag="psA", name="psA")
                    for q in range(HC):
                        te.matmul(pp[:], wsl(w2, q, hc),
                                  H1T[q][:, s512(ncj)],
                                  start=(q == 0), stop=(q == HC - 1))
                    if ncj % 2 == 0:
                        v.tensor_scalar(HmT[hc][:, s512(ncj)], pp[:],
                                        b2c[:, hc:hc + 1], None, OP.add)
                    else:
                        sc.activation(HmT[hc][:, s512(ncj)], pp[:], AF.Identity,
                                      bias=b2c[:, hc:hc + 1], scale=1.0)

            wk_w = load_w_dma(load_w("wk"), wk_d)

            # ---- phase 3b: HmB = H1 @ w2 (token-major, NO bias; f32r) ----
            # b2 is added to C_temp instead (softmax rows sum to 1).
            HmB_all = big.tile([128, 4 * N], f32r, tag="bigX", bufs=1,
                               name="HmB_all")
            for ncp in range(NCP):
                pp = psA.tile([128, 512], f32, tag="psA", name="psA")
                for q in range(HC):
                    te.matmul(pp[:], H1T[q][:, s128(ncp)],
                              w2[:, s512(q)],
                              start=(q == 0), stop=(q == HC - 1))
                dst = HmB_all[:, ncp * 512:(ncp + 1) * 512]
                if ncp % 2 == 0:
                    v.tensor_copy(dst, pp[:])
                else:
                    sc.copy(dst, pp[:])

            wq_w = load_w_dma(load_w("wq"), wq_d)

            # ---- phase 4: logitsT -> expT (=PT), expP, M, MT, inv_s ------
            PT = big.tile([128, N], f32r, tag="B", name="PT")
            g.memset(PT[64:128, :].bitcast(f32), 0.0)
            MT = big.tile([128, N], f32r, tag="B", name="MT")
            g.memset(MT[64:128, :].bitcast(f32), 0.0)
            inv_s = med.tile([128, NCP], f32, tag="invs")
            for ncj in range(NCJ):
                pl = psS.tile([128, 512], f32, tag="psS", name="psS")
                for hc in range(HC):
                    te.matmul(pl[:], EbarT[hc][:],
                              HmT[hc][:, s512(ncj)],
                              start=(hc == 0), stop=(hc == HC - 1))
                sc.activation(PT[0:64, s512(ncj)], pl[0:64, :], AF.Exp)
                mtp = psT.tile([128, 512], f32r, tag="pt512")
                for q in range(4):
                    ncp = ncj * 4 + q
                    # expP (token-major) via transpose of exp'd PT chunk
                    ep = psT.tile([128, 512], f32r, tag="pt512")
                    te.transpose(ep[:, 0:128], PT[:, s128(ncp)], identR[:])
                    s_col = sm.tile([128, 1], f32, tag="scol", bufs=2, name="scol")
                    v.reduce_sum(s_col[:], ep[:, 0:128].bitcast(f32), axis=AX)
                    v.reciprocal(inv_s[:, ncp:ncp + 1], s_col[:])
                    bs = sm.tile([128, K], f32, tag="bs", bufs=2, name="bs")
                    v.tensor_scalar(bs[:], bern[:, ncp * K:(ncp + 1) * K],
                                    s_col[:], None, OP.mult)
                    M = sm.tile([128, K], f32r, tag="M", bufs=2, name="M")
                    v.tensor_tensor(M[:], ep[:, 0:64].bitcast(f32), bs[:],
                                    OP.is_gt)
                    te.transpose(mtp[0:64, s128(q)], M[:], identR[:])
                if ncj % 2 == 0:
                    v.tensor_copy(MT[0:64, s512(ncj)], mtp[0:64, :])
                else:
                    sc.copy(MT[0:64, s512(ncj)], mtp[0:64, :])

            wout_w = load_w_dma(load_w("wout"), wout_d)

            # ---- phase 5: wkT (wk transposed); Kmat itself is never
            # materialized: scores = Q @ (Hm wk)^T = (Q wk^T) @ Hm^T -------
            wkT = load_w("wkT")
            for hc in range(HC):
                pt = psT.tile([128, 512], f32r, tag="pt512")
                for q in range(HC):
                    te.transpose(pt[:, s128(q)], wsl(wk_w, q, hc), identR[:])
                if hc % 2 == 0:
                    v.tensor_copy(wkT[:, hc * 512:(hc + 1) * 512], pt[:])
                else:
                    sc.copy(wkT[:, hc * 512:(hc + 1) * 512], pt[:])

            # ---- phase 6: Q (k-major, padded) -> QT chunks ---------------
            pq = psS.tile([128, 512], f32, tag="psS", name="psS")
            for q in range(HC):
                te.matmul(pq[:], ETp[q][:],
                          wq_w[:, s512(q)],
                          start=(q == 0), stop=(q == HC - 1))
            Qs = med.tile([128, 512], f32r, tag="Qs", name="Qs")
            v.scalar_tensor_tensor(Qs[:], pq[:], float(SCALE), wqb_bc[:],
                                   OP.mult, OP.add)
            QT = []
            for hc in range(HC):
                ptq = psT.tile([128, 512], f32r, tag="pt512")
                te.transpose(ptq[:, 0:128], Qs[:, s128(hc)], identR[:])
                t = med.tile([128, 128], f32r, tag=f"qt{hc}", name=f"qt{hc}")
                sc.copy(t[:], ptq[:, 0:128])
                QT.append(t)
            # Q2 = Q @ wk^T (k-major, padded rows), then Q2T chunks
            pq2 = psS.tile([128, 512], f32, tag="psS", name="psS")
            for hc in range(HC):
                te.matmul(pq2[:], QT[hc][:], wkT[:, hc * 512:(hc + 1) * 512],
                          start=(hc == 0), stop=(hc == HC - 1))
            Q2s = med.tile([128, 512], f32r, tag="Qs", name="Q2s")
            v.tensor_copy(Q2s[:], pq2[:])
            Q2T = []
            for hc in range(HC):
                ptq2 = psT.tile([128, 512], f32r, tag="pt512")
                te.transpose(ptq2[:, 0:128], Q2s[:, s128(hc)], identR[:])
                t2q = med.tile([128, 128], f32r, tag=f"qt{hc}", name=f"q2t{hc}")
                sc.copy(t2q[:], ptq2[:, 0:128])
                Q2T.append(t2q)

            # ---- phase 7: scores -> expS (zero-padded rows), row sums ----
            expS = big.tile([128, N], f32r, tag="B", name="expS")
            g.memset(expS[64:128, :].bitcast(f32), 0.0)
            pses = []
            for ncj in range(NCJ):
                ps_ = psS.tile([128, 512], f32, tag="psS", name="psS")
                for hc in range(HC):
                    te.matmul(ps_[:], Q2T[hc][:],
                              HmT[hc][:, s512(ncj)],
                              start=(hc == 0), stop=(hc == HC - 1))
                pse = med.tile([64, 1], f32, tag=f"pse{ncj}", name=f"pse{ncj}")
                sc.activation(expS[0:64, s512(ncj)], ps_[0:64, :], AF.Exp,
                              accum_out=pse[:])
                pses.append(pse)
            sA = med.tile([64, 1], f32, tag="sA")
            v.tensor_tensor(sA[:], pses[0][:], pses[1][:], OP.add)
            sA2 = med.tile([64, 1], f32, tag="sA2")
            v.tensor_tensor(sA2[:], pses[2][:], pses[3][:], OP.add)
            sA3 = med.tile([64, 1], f32, tag="sA3")
            v.tensor_tensor(sA3[:], sA[:], sA2[:], OP.add)
            invA = med.tile([64, 1], f32, tag="invA")
            v.reciprocal(invA[:], sA3[:])

            # expST: token-major chunks [128 tok, 128 kpad] (zero right cols)
            expST = big.tile([128, NCP * 128], f32r, tag="B", name="expST")
            for jj in range(4):
                pt = psT.tile([128, 512], f32r, tag="pt512")
                for q in range(4):
                    te.transpose(pt[:, s128(q)], expS[:, s128(jj * 4 + q)],
                                 identR[:])
                if jj % 2 == 0:
                    v.tensor_copy(expST[:, s512(jj)], pt[:])
                else:
                    sc.copy(expST[:, s512(jj)], pt[:])

            # ---- phase 8: Ctemp = (A @ Hm0) * invA + b2  (padded k) ------
            pc = psC.tile([128, 512], f32, tag="psC", name="psC")
            for ncp in range(NCP):
                te.matmul(pc[:], expST[:, s128(ncp)],
                          HmB_all[:, ncp * 512:(ncp + 1) * 512],
                          start=(ncp == 0), stop=(ncp == NCP - 1))
            Ctp = med.tile([128, H], f32r, tag="bern", name="Ctp")
            g.memset(Ctp[64:128, :].bitcast(f32), 0.0)
            v.scalar_tensor_tensor(Ctp[0:64, :], pc[0:64, :], invA[:],
                                   b2_bc[0:64, :], OP.mult, OP.add)

            # ---- phase 9: CpreT (+ norms on Scalar), then CpT right away
            # (CpT uses unscaled Ctp; the invn_bc scale rides the psum->cb
            # copy, so the PE keeps streaming while the norms finalize) ----
            CpreT = [big.tile([128, N], f32r, tag="D", name=f"CpreT{i}")
                     for i in range(HC)]
            invn = []
            for hc in range(HC):
                parts = []
                for ncj in range(NCJ):
                    pp = psA.tile([128, 512], f32, tag="psA", name="psA")
                    te.matmul(pp[:], Ctp[:, s128(hc)],
                              MT[:, s512(ncj)],
                              start=True, stop=True)
                    if ncj % 2 == 0:
                        v.tensor_copy(CpreT[hc][:, s512(ncj)], pp[:])
                    else:
                        sc.copy(CpreT[hc][:, s512(ncj)], pp[:])
                    sqs = sm.tile([128, 512], f32, tag="xsq", bufs=1, name="sqs")
                    prt = sm.tile([128, 1], f32, tag="cprt", bufs=4, name="cprt")
                    sc.activation(sqs[:], pp[:], AF.Square, accum_out=prt[:])
                    parts.append(prt)
                pa = sm.tile([128, 1], f32, tag="cpa", bufs=2, name="cpa")
                v.tensor_tensor(pa[:], parts[0][:], parts[1][:], OP.add)
                pb = sm.tile([128, 1], f32, tag="cpb", bufs=2, name="cpb")
                v.tensor_tensor(pb[:], parts[2][:], parts[3][:], OP.add)
                csq = med.tile([128, 1], f32, tag=f"csq{hc}", name=f"csq{hc}")
                v.tensor_tensor(csq[:], pa[:], pb[:], OP.add)
                cn = med.tile([128, 1], f32, tag=f"cn{hc}", name=f"cn{hc}")
                sc.sqrt(cn[:], csq[:])
                cn2 = med.tile([128, 1], f32, tag=f"cn2{hc}", name=f"cn2{hc}")
                v.tensor_scalar(cn2[:], cn[:], 1e-12, None, OP.max)
                iv = med.tile([128, 1], f32, tag=f"ivn{hc}", name=f"ivn{hc}")
                v.reciprocal(iv[:], cn2[:])
                invn.append(iv)

            # invn as broadcast tile for the token-major C scaling
            invnrow = rows[0:1, 1536:2048]
            for hc in range(HC):
                pt = psT.tile([128, 512], f32, tag="pt512")
                te.matmul(pt[0:1, 0:128], invn[hc][:, 0:1], ident[:],
                          start=True, stop=True)
                v.tensor_copy(invnrow[0:1, s128(hc)], pt[0:1, 0:128])
            ppn = psA.tile([128, 512], f32, tag="psA", name="psA")
            te.matmul(ppn[:], ones128[:], invnrow, start=True, stop=True)
            invn_bc = med.tile([128, H], f32, tag="wqbbc", name="invnbc")
            v.tensor_copy(invn_bc[:], ppn[:])

            # CpT (token-major) -> scaled C output, batched 1MB stores
            # staging reuses the big-A buffers (HmB is dead after phase 8)
            cb_all = big.tile([128, 4 * N], f32, tag="bigX", bufs=1,
                              name="cb_all")
            for j in range(NCJ):
                cb = cb_all[:, j * N:(j + 1) * N]
                for q in range(4):
                    ncp = j * 4 + q
                    pp = psA.tile([128, 512], f32, tag="psA", name="psA")
                    te.matmul(pp[:], MT[:, s128(ncp)],
                              Ctp[:], start=True, stop=True)
                    if q % 2 == 0:
                        v.tensor_tensor(cb[:, s512(q)], pp[:], invn_bc[:],
                                        OP.mult)
                    else:
                        sc.copy(cb[:, s512(q)], pp[:])
                        g.tensor_tensor(cb[:, s512(q)], cb[:, s512(q)],
                                        invn_bc[:], OP.mult)
                nc.sync.dma_start(
                    out=Co_d[s512(j), :].rearrange("(q p) c -> p q c", p=128),
                    in_=cb[:].rearrange("p (q c) -> p q c", q=4))

            # ---- phase 10: per-chunk fused tail: theta -> GT -> Y --------
            # thetaT_pre = E.T @ expT (unnormalized); GT = (CpreT*invn + HmT)
            # * thetaT_pre in place; Y = (GT.T @ wout) * inv_s + wout_b
            yb_all = big.tile([128, 4 * N], f32, tag="bigX", bufs=1,
                              name="yb_all")
            for j in range(NCJ):
                for hc in range(HC):
                    pth = psS.tile([128, 512], f32, tag="psS", name="psS")
                    te.matmul(pth[:], E_rP[:, s128(hc)],
                              PT[:, s512(j)], start=True, stop=True)
                    gslice = CpreT[hc][:, s512(j)]
                    ct = sm.tile([128, 512], f32, tag="ct", bufs=2, name="ct")
                    sc.activation(ct[:], gslice.bitcast(f32), AF.Identity,
                                  bias=0.0, scale=invn[hc][:])
                    if hc % 2 == 0:
                        v.tensor_tensor(ct[:], ct[:],
                                        HmT[hc][:, s512(j)].bitcast(f32),
                                        OP.add)
                    else:
                        g.tensor_tensor(ct[:], ct[:],
                                        HmT[hc][:, s512(j)].bitcast(f32),
                                        OP.add)
                    v.tensor_tensor(gslice, ct[:], pth[:], OP.mult)
                yb = yb_all[:, j * N:(j + 1) * N]
                for q in range(4):
                    ncp = j * 4 + q
                    pp = psA.tile([128, 512], f32, tag="psA", name="psA")
                    for hc in range(HC):
                        te.matmul(pp[:], CpreT[hc][:, s128(ncp)],
                                  wout_w[:, s512(hc)],
                                  start=(hc == 0), stop=(hc == HC - 1))
                    if q % 2 == 0:
                        v.scalar_tensor_tensor(yb[:, s512(q)], pp[:],
                                               inv_s[:, ncp:ncp + 1],
                                               wob_bc[:], OP.mult, OP.add)
                    else:
                        sc.activation(yb[:, s512(q)], pp[:], AF.Identity,
                                      bias=0.0, scale=inv_s[:, ncp:ncp + 1])
                        g.tensor_tensor(yb[:, s512(q)], yb[:, s512(q)],
                                        wob_bc[:], OP.add)
                for hh in range(2):
                    nc.sync.dma_start(
                        out=Y_d[j * 512 + hh * 256: j * 512 + (hh + 1) * 256, :]
                            .rearrange("(q p) c -> p q c", p=128),
                        in_=yb[:, hh * 1024:(hh + 1) * 1024]
                            .rearrange("p (q c) -> p q c", q=2))

    nc.finalize()
    return nc


def _get_nc():
    if "nc" not in _CACHE:
        _CACHE["nc"] = build_nc()
    return _CACHE["nc"]


def kernel(**inputs):
    from concourse.bass_utils import run_bass_kernel_spmd

    nc = _get_nc()
    arr = {k: np.ascontiguousarray(np.asarray(v, dtype=np.float32))
           for k, v in inputs.items()}
    shared = {k: arr[k] for k in
              ("cluster_embeddings", "mlp_w1", "mlp_b1", "mlp_w2", "mlp_b2",
               "wq", "wq_b", "wk", "wk_b", "wout", "wout_b")}
    in_maps = [dict(x=arr["x"][b], bern_u=arr["bern_u"][b], **shared)
               for b in range(B)]
    res = run_bass_kernel_spmd(nc, in_maps, list(range(B))).results
    Y = np.stack([res[b]["Y"] for b in range(B)])
    Co = np.stack([res[b]["C_out"] for b in range(B)])
    return (Y, Co)


if __name__ == "__main__":
    import os
    os.environ.setdefault("JAX_PLATFORMS", "cpu")
    from concourse.timeline_sim import TimelineSim

    nc = build_nc()
    ts = TimelineSim(nc, trace=False)
    print("TimelineSim:", ts.simulate(), "ns")


# revision 46
# speedup vs baseline: 1.2577x; 1.0511x over previous
"""Trainium2 Bass kernel for nn_CCM_73985106641118 (vq_codebook).

Data-parallel across the batch dim: core b processes batch b (8 cores, B=8).

Layout strategy: activations live feature-major ([feature chunk -> 128
partitions, tokens -> free dim]) wherever possible; each GEMM picks its
stationary operand so the output lands in the orientation its consumer needs.
Hm is produced in BOTH orientations by two GEMMs from H1T (cheaper than PE
transposes). All matmuls run fp32r (1 cyc/row when the moving free dim >= 256,
vs 4 cyc/row for fp32); fp32r operands must be produced as fp32r, so
producers write fp32r directly and non-PE readers bitcast back to f32.
Operands with a 64-sized partition dim (the K=64 cluster axis) are
zero-padded to 128 partitions (measured ~2.6x penalty for 64-dim matmuls).

Algebraic simplifications vs the reference:
 - wk_b adds a per-cluster constant to the attention scores; softmax over
   tokens is shift-invariant per row, so wk_b is dropped entirely.
 - Cluster softmax P is kept unnormalized (expP); the 1/sum factor folds
   into the Y epilogue as a per-token scale, and the bernoulli compare uses
   bern*s < expP.
 - A @ (Hm0 + b2) = A @ Hm0 + b2 because softmax rows sum to 1, so HmB is
   built bias-free and b2 is added once to C_temp.

Scheduling: the PE's HAM throttles to half speed after idle windows, so
phases are interleaved to keep the PE streaming (x-normalize/transpose fused
with GEMM1 per 512-token block; the tail runs CpT while the C norms
finalize, then theta/GT/Y per block). DMA issue queues are split: x chunks
and output stores on SP, weight loads on the Scalar DGE, small loads on the
Vector DGE. Elementwise work is spread across Scalar (activations with
accum_out for sums-of-squares), Vector, and Pool (SBUF->SBUF only; Pool has
no PSUM port).
"""

import numpy as np

import concourse.bacc as bacc
import concourse.mybir as mybir
from concourse.masks import make_identity
from concourse.tile import TileContext

f32 = mybir.dt.float32
f32r = mybir.dt.float32r
AX = mybir.AxisListType.X
OP = mybir.AluOpType
AF = mybir.ActivationFunctionType

B, N, C, H, K = 8, 2048, 512, 512, 64
NCP = N // 128   # 16 token chunks of 128
NCJ = N // 512   # 4 token chunks of 512
HC = H // 128    # 4 feature chunks of 128
SCALE = 1.0 / np.sqrt(np.float32(H))

_CACHE = {}


def s128(i):
    return slice(i * 128, (i + 1) * 128)


def s512(i):
    return slice(i * 512, (i + 1) * 512)


def build_nc():
    nc = bacc.Bacc("TRN2", target_bir_lowering=False, debug=False, num_devices=8)

    x_d = nc.dram_tensor("x", [N, C], f32, kind="ExternalInput").ap()
    bu_d = nc.dram_tensor("bern_u", [N, K], f32, kind="ExternalInput").ap()
    E_d = nc.dram_tensor("cluster_embeddings", [K, H], f32, kind="ExternalInput").ap()
    w1_d = nc.dram_tensor("mlp_w1", [C, H], f32, kind="ExternalInput").ap()
    b1_d = nc.dram_tensor("mlp_b1", [H], f32, kind="ExternalInput").ap()
    w2_d = nc.dram_tensor("mlp_w2", [H, H], f32, kind="ExternalInput").ap()
    b2_d = nc.dram_tensor("mlp_b2", [H], f32, kind="ExternalInput").ap()
    wq_d = nc.dram_tensor("wq", [H, H], f32, kind="ExternalInput").ap()
    wqb_d = nc.dram_tensor("wq_b", [H], f32, kind="ExternalInput").ap()
    wk_d = nc.dram_tensor("wk", [H, H], f32, kind="ExternalInput").ap()
    nc.dram_tensor("wk_b", [H], f32, kind="ExternalInput")  # unused (see header)
    wout_d = nc.dram_tensor("wout", [H, C], f32, kind="ExternalInput").ap()
    woutb_d = nc.dram_tensor("wout_b", [C], f32, kind="ExternalInput").ap()
    Y_d = nc.dram_tensor("Y", [N, C], f32, kind="ExternalOutput").ap()
    Co_d = nc.dram_tensor("C_out", [N, H], f32, kind="ExternalOutput").ap()

    with TileContext(nc) as tc:
        with (
            tc.tile_pool(name="big", bufs=4) as big,
            tc.tile_pool(name="med", bufs=1) as med,
            tc.tile_pool(name="sm", bufs=3) as sm,
            tc.tile_pool(name="psA", bufs=3, space="PSUM") as psA,
            tc.tile_pool(name="psT", bufs=2, space="PSUM") as psT,
            tc.tile_pool(name="psS", bufs=2, space="PSUM") as psS,
            tc.tile_pool(name="psC", bufs=1, space="PSUM") as psC,
        ):
            v = nc.vector
            sc = nc.scalar
            te = nc.tensor
            g = nc.gpsimd

            # identities first: nothing on the Pool/Vector queues ahead of
            # them, so the PE warm-up below can start at ~1.5us
            ident = med.tile([128, 128], f32, tag="ident")
            make_identity(nc, ident[:])
            identR = med.tile([128, 128], f32r, tag="rowsR")
            v.tensor_copy(identR[:], ident[:])

            # PE warm-up: the HAM throttles a cold/idle PE to 0.65-1.2 GHz;
            # stream dummy matmuls (never read) while the first x chunks are
            # still in flight so phase 1 starts at full clock
            # PE warm-up: the HAM throttles a cold/idle PE to 0.65-1.2
            # GHz; stream dummy matmuls (never read) while the first x
            # chunks are in flight so phase 1 starts at full clock. The
            # warm tile is the first psS allocation and is dead before any
            # real psS user.
            warm = psS.tile([128, 512], f32, tag="psS", name="warm")
            for _ in range(24):
                te.matmul(warm[:, 0:128], identR[:], identR[:],
                          start=True, stop=True)

            # ---- x chunk staging on the SP queue (its dedicated queue) ---
            def xstage(ncp):
                t = sm.tile([128, 512], f32, tag="xq", bufs=3, name=f"xq{ncp}")
                nc.sync.dma_start(out=t[:], in_=x_d[s128(ncp), :])
                return t

            xq_tiles = {ncp: xstage(ncp) for ncp in range(3)}

            # weight loads on the Scalar DGE queue: DMA quarters into an f32
            # stage, Pool CASTs into the f32r tile. [128, 2048]: row block q
            # of the [512, 512] weight lives at cols [q*512, (q+1)*512).
            # 3 buffers: w1,w2,wk live together; wq reuses w1's buffer after
            # GEMM1, wout reuses w2's after phase 3b (loads deferred there).
            def load_w(name):
                return med.tile([128, N], f32r, tag="W", bufs=3, name=name)

            def load_w_dma(t, dram, engs=(g, g, g, g)):
                # two half DMAs through a single stage buffer; CASTs into the
                # f32r tile are split across engines. Each load_w_dma call is
                # placed in program order right before the phase that needs
                # the weight, so the stage-buffer wait never blocks the
                # Scalar queue during head compute.
                for h in range(2):
                    st = sm.tile([128, 1024], f32, tag="wst", bufs=1,
                                 name=f"wst{h}")
                    sc.dma_start(
                        out=st[:].rearrange("p (q h) -> p q h", q=2),
                        in_=dram[h * 256:(h + 1) * 256, :]
                            .rearrange("(q p) h -> p q h", p=128))
                    for k in range(2):
                        eng = engs[h * 2 + k]
                        dst = t[:, (h * 2 + k) * 512:(h * 2 + k + 1) * 512]
                        if eng is sc:
                            sc.copy(dst, st[:, k * 512:(k + 1) * 512])
                        else:
                            eng.tensor_copy(dst, st[:, k * 512:(k + 1) * 512])
                return t

            E_f = med.tile([64, H], f32, tag="E")
            g.dma_start(out=E_f[:], in_=E_d[:, :])
            bern = med.tile([128, NCP * K], f32, tag="bern")
            g.dma_start(out=bern[:].rearrange("p (q k) -> p q k", q=16),
                         in_=bu_d.rearrange("(q p) k -> p q k", p=128))

            w1 = load_w_dma(load_w("w1"), w1_d, engs=(v, sc, v, sc))

            def bias_cols(dram, tag):
                t = med.tile([128, HC], f32, tag=tag, name=tag)
                g.dma_start(out=t[:], in_=dram.rearrange("(j p) -> p j", p=128))
                return t

            b1c = bias_cols(b1_d, "b1c")
            b2c = bias_cols(b2_d, "b2c")

            # all [1, 512] bias rows packed into one [1, 2048] tile
            rows = med.tile([1, 2048], f32, tag="rows")
            b2row = rows[0:1, 0:512]
            g.dma_start(out=b2row, in_=b2_d.rearrange("(o a) -> o a", o=1))
            wqbrow = rows[0:1, 512:1024]
            g.dma_start(out=wqbrow, in_=wqb_d.rearrange("(o a) -> o a", o=1))
            wobrow = rows[0:1, 1024:1536]
            g.dma_start(out=wobrow, in_=woutb_d.rearrange("(o a) -> o a", o=1))

            # weight slice: row block q (contraction chunk), col chunk hc
            def wsl(t, q, hc):
                return t[:, q * 512 + hc * 128: q * 512 + (hc + 1) * 128]


            ones128 = med.tile([1, 128], f32, tag="ones")
            g.memset(ones128[:], 1.0)

            def bcast_row(row, tag):
                pp = psA.tile([128, 512], f32, tag="psA", name="psA")
                te.matmul(pp[:], ones128[:], row, start=True, stop=True)
                t = med.tile([128, 512], f32, tag="bcast", bufs=2, name=tag)
                v.tensor_copy(t[:], pp[:])
                return t

            b2_bc = bcast_row(b2row, "b2bc")
            wob_bc = bcast_row(wobrow, "wobbc")
            # wqb broadcast pre-scaled by 1/sqrt(H)
            ppq = psA.tile([128, 512], f32, tag="psA", name="psA")
            te.matmul(ppq[:], ones128[:], wqbrow, start=True, stop=True)
            wqb_bc = med.tile([128, 512], f32, tag="wqbbc")
            v.tensor_scalar(wqb_bc[:], ppq[:], float(SCALE), None, OP.mult)

            # ---- E prep: norms, Ebar, padded transposes ------------------
            esq = med.tile([64, H], f32, tag="Qs", name="esq")
            ensq = med.tile([64, 1], f32, tag="ensq")
            sc.activation(esq[:], E_f[:], AF.Square, accum_out=ensq[:])
            enrm = med.tile([64, 1], f32, tag="enrm")
            sc.sqrt(enrm[:], ensq[:])
            einv = med.tile([64, 1], f32, tag="einv")
            v.reciprocal(einv[:], enrm[:])
            Ebar = med.tile([64, H], f32, tag="Ebar")
            v.tensor_scalar(Ebar[:], E_f[:], einv[:], None, OP.mult)

            # E_rPad: [128, 512] E on top, zeros below (theta stationary)
            E_rP = med.tile([128, H], f32r, tag="ErP")
            g.memset(E_rP[64:128, :].bitcast(f32), 0.0)
            g.tensor_copy(E_rP[0:64, :], E_f[:])

            # EbarT / ET chunks padded to [128, 128] (zero cols 64..127)
            EbarT, ETp = [], []
            for hc in range(HC):
                t = med.tile([128, 128], f32r, tag=f"ebt{hc}", name=f"ebt{hc}")
                g.memset(t[:, 64:128].bitcast(f32), 0.0)
                pt = psT.tile([128, 512], f32, tag="pt512")
                te.transpose(pt[0:128, 0:64], Ebar[:, s128(hc)], ident[0:64, 0:64])
                sc.copy(t[:, 0:64], pt[0:128, 0:64])
                EbarT.append(t)
                t2 = med.tile([128, 128], f32r, tag=f"et{hc}", name=f"et{hc}")
                g.memset(t2[:, 64:128].bitcast(f32), 0.0)
                pt2 = psT.tile([128, 512], f32, tag="pt512")
                te.transpose(pt2[0:128, 0:64], E_f[:, s128(hc)], ident[0:64, 0:64])
                sc.copy(t2[:, 0:64], pt2[0:128, 0:64])
                ETp.append(t2)

            # ---- phase 1+2 fused per 512-token block: l2norm + transpose,
            # then H1T = relu(w1.T @ xnT + b1) for the block  (f32r) -------
            # xnT_all[:, cc*2048 + n] holds feature chunk cc, token n
            xnT_all = big.tile([128, 4 * N], f32r, tag="bigX", bufs=1,
                               name="xnT_all")

            def xnT(cc):
                return xnT_all[:, cc * N:(cc + 1) * N]

            H1T = [big.tile([128, N], f32r, tag="B", name=f"H1T{i}") for i in range(HC)]
            for j in range(NCJ):
                for q in range(4):
                    ncp = j * 4 + q
                    xq = xq_tiles[ncp] if ncp < 3 else xstage(ncp)
                    xt = xq[:]
                    xsq = sm.tile([128, C], f32, tag="xsq", bufs=1, name="xsq")
                    ssq = sm.tile([128, 1], f32, tag="ssq", bufs=2, name="ssq")
                    sc.activation(xsq[:], xt, AF.Square, accum_out=ssq[:])
                    nrm = sm.tile([128, 1], f32, tag="nrm", bufs=2, name="nrm")
                    sc.sqrt(nrm[:], ssq[:])
                    nrm2 = sm.tile([128, 1], f32, tag="nrm2", bufs=2, name="nrm2")
                    v.tensor_scalar(nrm2[:], nrm[:], 1e-12, None, OP.max)
                    inv = sm.tile([128, 1], f32, tag="inv", bufs=2, name="inv")
                    v.reciprocal(inv[:], nrm2[:])
                    xn = sm.tile([128, C], f32r, tag="xn", bufs=2, name="xn")
                    v.tensor_scalar(xn[:], xt, inv[:], None, OP.mult)
                    # 4 transposes into one PSUM bank, one strided copy out
                    pt = psT.tile([128, 512], f32r, tag="pt512")
                    for cc in range(HC):
                        te.transpose(pt[:, s128(cc)], xn[:, s128(cc)],
                                     identR[:])
                    dst = xnT_all[:].rearrange(
                        "p (c n) -> p c n", c=4)[:, :, ncp * 128:(ncp + 1) * 128]
                    if ncp % 2 == 0:
                        v.tensor_copy(dst, pt[:].rearrange("p (c n) -> p c n", c=4))
                    else:
                        sc.copy(dst, pt[:].rearrange("p (c n) -> p c n", c=4))
                # GEMM1 for this 512-token block
                for h1c in range(HC):
                    pp = psA.tile([128, 512], f32, tag="psA", name="psA")
                    for cc in range(HC):
                        te.matmul(pp[:], wsl(w1, cc, h1c),
                                  xnT(cc)[:, s512(j)],
                                  start=(cc == 0), stop=(cc == HC - 1))
                    sc.activation(H1T[h1c][:, s512(j)], pp[:], AF.Relu,
                                  bias=b1c[:, h1c:h1c + 1], scale=1.0)


            # w2 load deferred here: its stage wait no longer blocks the
            # Scalar queue during the head
            w2 = load_w_dma(load_w("w2"), w2_d)

            # ---- phase 3a: HmT = w2.T @ H1T + b2 (feature-major, f32r) ---
            HmT = [big.tile([128, N], f32r, tag="C", name=f"HmT{i}") for i in range(HC)]
            for hc in range(HC):
                for ncj in range(NCJ):
                    pp = psA.tile([128, 512], f32, tag="psA", name="psA")
                    for q in range(HC):
                        te.matmul(pp[:], wsl(w2, q, hc),
                                  H1T[q][:, s512(ncj)],
                                  start=(q == 0), stop=(q == HC - 1))
                    if ncj % 2 == 0:
                        v.tensor_scalar(HmT[hc][:, s512(ncj)], pp[:],
                                        b2c[:, hc:hc + 1], None, OP.add)
                    else:
                        sc.activation(HmT[hc][:, s512(ncj)], pp[:], AF.Identity,
                                      bias=b2c[:, hc:hc + 1], scale=1.0)

            wk_w = load_w_dma(load_w("wk"), wk_d)

            # ---- phase 3b: HmB = H1 @ w2 (token-major, NO bias; f32r) ----
            # b2 is added to C_temp instead (softmax rows sum to 1).
            HmB_all = big.tile([128, 4 * N], f32r, tag="bigX", bufs=1,
                               name="HmB_all")
            for ncp in range(NCP):
                pp = psA.tile([128, 512], f32, tag="psA", name="psA")
                for q in range(HC):
                    te.matmul(pp[:], H1T[q][:, s128(ncp)],
                              w2[:, s512(q)],
                              start=(q == 0), stop=(q == HC - 1))
                dst = HmB_all[:, ncp * 512:(ncp + 1) * 512]
                if ncp % 2 == 0:
                    v.tensor_copy(dst, pp[:])
                else:
                    sc.copy(dst, pp[:])

            wq_w = load_w_dma(load_w("wq"), wq_d)

            # ---- phase 4: logitsT -> expT (=PT), expP, M, MT, inv_s ------
            PT = big.tile([128, N], f32r, tag="B", name="PT")
            g.memset(PT[64:128, :].bitcast(f32), 0.0)
            MT = big.tile([128, N], f32r, tag="B", name="MT")
            g.memset(MT[64:128, :].bitcast(f32), 0.0)
            inv_s = med.tile([128, NCP], f32, tag="invs")
            for ncj in range(NCJ):
                pl = psS.tile([128, 512], f32, tag="psS", name="psS")
                for hc in range(HC):
                    te.matmul(pl[:], EbarT[hc][:],
                              HmT[hc][:, s512(ncj)],
                              start=(hc == 0), stop=(hc == HC - 1))
                sc.activation(PT[0:64, s512(ncj)], pl[0:64, :], AF.Exp)
                mtp = psT.tile([128, 512], f32r, tag="pt512")
                for q in range(4):
                    ncp = ncj * 4 + q
                    # expP (token-major) via transpose of exp'd PT chunk
                    ep = psT.tile([128, 512], f32r, tag="pt512")
                    te.transpose(ep[:, 0:128], PT[:, s128(ncp)], identR[:])
                    s_col = sm.tile([128, 1], f32, tag="scol", bufs=2, name="scol")
                    v.reduce_sum(s_col[:], ep[:, 0:128].bitcast(f32), axis=AX)
                    v.reciprocal(inv_s[:, ncp:ncp + 1], s_col[:])
                    bs = sm.tile([128, K], f32, tag="bs", bufs=1, name="bs")
                    v.tensor_scalar(bs[:], bern[:, ncp * K:(ncp + 1) * K],
                                    s_col[:], None, OP.mult)
                    M = sm.tile([128, K], f32r, tag="M", bufs=2, name="M")
                    v.tensor_tensor(M[:], ep[:, 0:64].bitcast(f32), bs[:],
                                    OP.is_gt)
                    te.transpose(mtp[0:64, s128(q)], M[:], identR[:])
                if ncj % 2 == 0:
                    v.tensor_copy(MT[0:64, s512(ncj)], mtp[0:64, :])
                else:
                    sc.copy(MT[0:64, s512(ncj)], mtp[0:64, :])

            wout_w = load_w_dma(load_w("wout"), wout_d)

            # ---- phase 5: wkT (wk transposed); Kmat itself is never
            # materialized: scores = Q @ (Hm wk)^T = (Q wk^T) @ Hm^T -------
            wkT = load_w("wkT")
            for hc in range(HC):
                pt = psT.tile([128, 512], f32r, tag="pt512")
                for q in range(HC):
                    te.transpose(pt[:, s128(q)], wsl(wk_w, q, hc), identR[:])
                if hc % 2 == 0:
                    v.tensor_copy(wkT[:, hc * 512:(hc + 1) * 512], pt[:])
                else:
                    sc.copy(wkT[:, hc * 512:(hc + 1) * 512], pt[:])

            # ---- phase 6: Q (k-major, padded) -> QT chunks ---------------
            pq = psS.tile([128, 512], f32, tag="psS", name="psS")
            for q in range(HC):
                te.matmul(pq[:], ETp[q][:],
                          wq_w[:, s512(q)],
                          start=(q == 0), stop=(q == HC - 1))
            Qs = med.tile([128, 512], f32r, tag="Qs", name="Qs")
            v.scalar_tensor_tensor(Qs[:], pq[:], float(SCALE), wqb_bc[:],
                                   OP.mult, OP.add)
            QT = []
            for hc in range(HC):
                ptq = psT.tile([128, 512], f32r, tag="pt512")
                te.transpose(ptq[:, 0:128], Qs[:, s128(hc)], identR[:])
                t = med.tile([128, 128], f32r, tag=f"qt{hc}", name=f"qt{hc}")
                sc.copy(t[:], ptq[:, 0:128])
                QT.append(t)
            # Q2 = Q @ wk^T (k-major, padded rows), then Q2T chunks
            pq2 = psS.tile([128, 512], f32, tag="psS", name="psS")
            for hc in range(HC):
                te.matmul(pq2[:], QT[hc][:], wkT[:, hc * 512:(hc + 1) * 512],
                          start=(hc == 0), stop=(hc == HC - 1))
            Q2s = med.tile([128, 512], f32r, tag="Qs", name="Q2s")
            v.tensor_copy(Q2s[:], pq2[:])
            Q2T = []
            for hc in range(HC):
                ptq2 = psT.tile([128, 512], f32r, tag="pt512")
                te.transpose(ptq2[:, 0:128], Q2s[:, s128(hc)], identR[:])
                t2q = med.tile([128, 128], f32r, tag=f"qt{hc}", name=f"q2t{hc}")
                sc.copy(t2q[:], ptq2[:, 0:128])
                Q2T.append(t2q)

            # ---- phase 7: scores -> expS (zero-padded rows), row sums ----
            expS = big.tile([128, N], f32r, tag="B", name="expS")
            g.memset(expS[64:128, :].bitcast(f32), 0.0)
            pses = []
            for ncj in range(NCJ):
                ps_ = psS.tile([128, 512], f32, tag="psS", name="psS")
                for hc in range(HC):
                    te.matmul(ps_[:], Q2T[hc][:],
                              HmT[hc][:, s512(ncj)],
                              start=(hc == 0), stop=(hc == HC - 1))
                pse = med.tile([64, 1], f32, tag=f"pse{ncj}", name=f"pse{ncj}")
                sc.activation(expS[0:64, s512(ncj)], ps_[0:64, :], AF.Exp,
                              accum_out=pse[:])
                pses.append(pse)
            sA = med.tile([64, 1], f32, tag="sA")
            v.tensor_tensor(sA[:], pses[0][:], pses[1][:], OP.add)
            sA2 = med.tile([64, 1], f32, tag="sA2")
            v.tensor_tensor(sA2[:], pses[2][:], pses[3][:], OP.add)
            sA3 = med.tile([64, 1], f32, tag="sA3")
            v.tensor_tensor(sA3[:], sA[:], sA2[:], OP.add)
            invA = med.tile([64, 1], f32, tag="invA")
            v.reciprocal(invA[:], sA3[:])

            # expST: token-major chunks [128 tok, 128 kpad] (zero right cols)
            expST = big.tile([128, NCP * 128], f32r, tag="B", name="expST")
            for jj in range(4):
                pt = psT.tile([128, 512], f32r, tag="pt512")
                for q in range(4):
                    te.transpose(pt[:, s128(q)], expS[:, s128(jj * 4 + q)],
                                 identR[:])
                if jj % 2 == 0:
                    v.tensor_copy(expST[:, s512(jj)], pt[:])
                else:
                    sc.copy(expST[:, s512(jj)], pt[:])

            # ---- phase 8: Ctemp = (A @ Hm0) * invA + b2  (padded k) ------
            pc = psC.tile([128, 512], f32, tag="psC", name="psC")
            for ncp in range(NCP):
                te.matmul(pc[:], expST[:, s128(ncp)],
                          HmB_all[:, ncp * 512:(ncp + 1) * 512],
                          start=(ncp == 0), stop=(ncp == NCP - 1))
            Ctp = med.tile([128, H], f32r, tag="bern", name="Ctp")
            g.memset(Ctp[64:128, :].bitcast(f32), 0.0)
            v.scalar_tensor_tensor(Ctp[0:64, :], pc[0:64, :], invA[:],
                                   b2_bc[0:64, :], OP.mult, OP.add)

            # ---- phase 9: CpreT (+ norms on Scalar), then CpT right away
            # (CpT uses unscaled Ctp; the invn_bc scale rides the psum->cb
            # copy, so the PE keeps streaming while the norms finalize) ----
            CpreT = [big.tile([128, N], f32r, tag="D", name=f"CpreT{i}")
                     for i in range(HC)]
            invn = []
            for hc in range(HC):
                parts = []
                for ncj in range(NCJ):
                    pp = psA.tile([128, 512], f32, tag="psA", name="psA")
                    te.matmul(pp[:], Ctp[:, s128(hc)],
                              MT[:, s512(ncj)],
                              start=True, stop=True)
                    if ncj % 2 == 0:
                        v.tensor_copy(CpreT[hc][:, s512(ncj)], pp[:])
                    else:
                        sc.copy(CpreT[hc][:, s512(ncj)], pp[:])
                    sqs = sm.tile([128, 512], f32, tag="xsq", bufs=1, name="sqs")
                    prt = sm.tile([128, 1], f32, tag="cprt", bufs=4, name="cprt")
                    sc.activation(sqs[:], pp[:], AF.Square, accum_out=prt[:])
                    parts.append(prt)
                pa = sm.tile([128, 1], f32, tag="cpa", bufs=2, name="cpa")
                v.tensor_tensor(pa[:], parts[0][:], parts[1][:], OP.add)
                pb = sm.tile([128, 1], f32, tag="cpb", bufs=2, name="cpb")
                v.tensor_tensor(pb[:], parts[2][:], parts[3][:], OP.add)
                csq = med.tile([128, 1], f32, tag=f"csq{hc}", name=f"csq{hc}")
                v.tensor_tensor(csq[:], pa[:], pb[:], OP.add)
                cn = med.tile([128, 1], f32, tag=f"cn{hc}", name=f"cn{hc}")
                sc.sqrt(cn[:], csq[:])
                cn2 = med.tile([128, 1], f32, tag=f"cn2{hc}", name=f"cn2{hc}")
                v.tensor_scalar(cn2[:], cn[:], 1e-12, None, OP.max)
                iv = med.tile([128, 1], f32, tag=f"ivn{hc}", name=f"ivn{hc}")
                v.reciprocal(iv[:], cn2[:])
                invn.append(iv)

            # invn as broadcast tile for the token-major C scaling
            invnrow = rows[0:1, 1536:2048]
            for hc in range(HC):
                pt = psT.tile([128, 512], f32, tag="pt512")
                te.matmul(pt[0:1, 0:128], invn[hc][:, 0:1], ident[:],
                          start=True, stop=True)
                v.tensor_copy(invnrow[0:1, s128(hc)], pt[0:1, 0:128])
            ppn = psA.tile([128, 512], f32, tag="psA", name="psA")
            te.matmul(ppn[:], ones128[:], invnrow, start=True, stop=True)
            invn_bc = med.tile([128, H], f32, tag="wqbbc", name="invnbc")
            v.tensor_copy(invn_bc[:], ppn[:])

            # CpT (token-major) -> scaled C output, batched 1MB stores
            # staging reuses the big-A buffers (HmB is dead after phase 8)
            cb_all = big.tile([128, 4 * N], f32, tag="bigX", bufs=1,
                              name="cb_all")
            for j in range(NCJ):
                cb = cb_all[:, j * N:(j + 1) * N]
                for q in range(4):
                    ncp = j * 4 + q
                    pp = psA.tile([128, 512], f32, tag="psA", name="psA")
                    te.matmul(pp[:], MT[:, s128(ncp)],
                              Ctp[:], start=True, stop=True)
                    if q % 2 == 0:
                        v.tensor_tensor(cb[:, s512(q)], pp[:], invn_bc[:],
                                        OP.mult)
                    else:
                        sc.copy(cb[:, s512(q)], pp[:])
                        g.tensor_tensor(cb[:, s512(q)], cb[:, s512(q)],
                                        invn_bc[:], OP.mult)
                nc.sync.dma_start(
                    out=Co_d[s512(j), :].rearrange("(q p) c -> p q c", p=128),
                    in_=cb[:].rearrange("p (q c) -> p q c", q=4))

            # ---- phase 10: per-chunk fused tail: theta -> GT -> Y --------
            # thetaT_pre = E.T @ expT (unnormalized); GT = (CpreT*invn + HmT)
            # * thetaT_pre in place; Y = (GT.T @ wout) * inv_s + wout_b
            yb_all = big.tile([128, 4 * N], f32, tag="bigX", bufs=1,
                              name="yb_all")
            for j in range(NCJ):
                for hc in range(HC):
                    pth = psS.tile([128, 512], f32, tag="psS", name="psS")
                    te.matmul(pth[:], E_rP[:, s128(hc)],
                              PT[:, s512(j)], start=True, stop=True)
                    gslice = CpreT[hc][:, s512(j)]
                    ct = sm.tile([128, 512], f32, tag="ct", bufs=2, name="ct")
                    sc.activation(ct[:], gslice.bitcast(f32), AF.Identity,
                                  bias=0.0, scale=invn[hc][:])
                    if hc % 2 == 0:
                        v.tensor_tensor(ct[:], ct[:],
                                        HmT[hc][:, s512(j)].bitcast(f32),
                                        OP.add)
                    else:
                        g.tensor_tensor(ct[:], ct[:],
                                        HmT[hc][:, s512(j)].bitcast(f32),
                                        OP.add)
                    v.tensor_tensor(gslice, ct[:], pth[:], OP.mult)
                yb = yb_all[:, j * N:(j + 1) * N]
                for q in range(4):
                    ncp = j * 4 + q
                    pp = psA.tile([128, 512], f32, tag="psA", name="psA")
                    for hc in range(HC):
                        te.matmul(pp[:], CpreT[hc][:, s128(ncp)],
                                  wout_w[:, s512(hc)],
                                  start=(hc == 0), stop=(hc == HC - 1))
                    if q % 2 == 0:
                        v.scalar_tensor_tensor(yb[:, s512(q)], pp[:],
                                               inv_s[:, ncp:ncp + 1],
                                               wob_bc[:], OP.mult, OP.add)
                    else:
                        sc.activation(yb[:, s512(q)], pp[:], AF.Identity,
                                      bias=0.0, scale=inv_s[:, ncp:ncp + 1])
                        g.tensor_tensor(yb[:, s512(q)], yb[:, s512(q)],
                                        wob_bc[:], OP.add)
                for hh in range(2):
                    nc.sync.dma_start(
                        out=Y_d[j * 512 + hh * 256: j * 512 + (hh + 1) * 256, :]
                            .rearrange("(q p) c -> p q c", p=128),
                        in_=yb[:, hh * 1024:(hh + 1) * 1024]
                            .rearrange("p (q c) -> p q c", q=2))

    nc.finalize()
    return nc


def _get_nc():
    if "nc" not in _CACHE:
        _CACHE["nc"] = build_nc()
    return _CACHE["nc"]


def kernel(**inputs):
    from concourse.bass_utils import run_bass_kernel_spmd

    nc = _get_nc()
    arr = {k: np.ascontiguousarray(np.asarray(v, dtype=np.float32))
           for k, v in inputs.items()}
    shared = {k: arr[k] for k in
              ("cluster_embeddings", "mlp_w1", "mlp_b1", "mlp_w2", "mlp_b2",
               "wq", "wq_b", "wk", "wk_b", "wout", "wout_b")}
    in_maps = [dict(x=arr["x"][b], bern_u=arr["bern_u"][b], **shared)
               for b in range(B)]
    res = run_bass_kernel_spmd(nc, in_maps, list(range(B))).results
    Y = np.stack([res[b]["Y"] for b in range(B)])
    Co = np.stack([res[b]["C_out"] for b in range(B)])
    return (Y, Co)


if __name__ == "__main__":
    import os
    os.environ.setdefault("JAX_PLATFORMS", "cpu")
    from concourse.timeline_sim import TimelineSim

    nc = build_nc()
    ts = TimelineSim(nc, trace=False)
    print("TimelineSim:", ts.simulate(), "ns")
